# revision 10
# baseline (speedup 1.0000x reference)
"""Trainium2 Bass kernel for nn_CoeusBlock (dense transformer block with
GQA causal attention, log-space scan recurrence, contrastive logic gate,
and depth-2 recursive gated FFN).

Sharding: 8 cores = 2 batches x 4 token-chunks of 512. Uniform SPMD via a
sequence-rotation trick: each core receives its batch rotated to
[future, prefix, own] order so its own tokens are always the last 4
chunks of 16 -> identical program on every core, only data differs.
Future tokens are neutralized in attention by an exp-bias penalty and in
the scan by forcing e = exp(u - 30000) = 0 so the zero-initialized state
passes through them unchanged.

All large matmuls run in bf16 (fp32 PSUM accumulate). Norm weights are
folded into adjacent weight matrices on the host. The log-space scan
  log_h = a* + cumlogsumexp(u - a*)
is computed as the equivalent linear recurrence h[t] = f[t]*h[t-1]+e[t]
(f = sigmoid(pre), e = exp(u)) with DVE tensor_tensor_scan per feature
chunk - numerically safe for this operator's value ranges (verified).
Per-token scale rows ([1,T], e.g. 1/rms) are replicated across the 128
partitions with a 1-contraction ones matmul before elementwise use.
"""

import numpy as np
import ml_dtypes

import concourse.bass as bass
import concourse.mybir as mybir
import concourse.tile as tile
from concourse import bacc
from concourse.bass_utils import run_bass_kernel_spmd

# ---- problem constants (hardcoded per contract) ----
B, T, D = 2, 2048, 1024
H, KV, HD = 16, 4, 64
HID = 3072
DEPTH = 2
EPS_RMS = 1e-6
EPS_LN = 1e-5
P = 128
DC = D // P                   # 8 feature chunks
NCH = T // P                  # 16 token chunks
TO = T // 4                   # 512 own tokens per core
OC = TO // P                  # 4 own token chunks
HC = HID // P                 # 24
PEN = -30000.0

F32 = mybir.dt.float32
BF16 = mybir.dt.bfloat16
AL = mybir.AluOpType
AF = mybir.ActivationFunctionType

_CACHE = {}


def _build(debug=False):
    nc = bacc.Bacc(None, target_bir_lowering=False, debug=debug)

    def din(name, shape, dt=F32):
        return nc.dram_tensor(name, list(shape), dt, kind="ExternalInput")

    x_d = din("x", [P, DC, T])                    # rotated x^T
    xq_d = din("xq", [P, DC, TO])                 # own x^T (orig order)
    wq_d = din("wq", [P, DC, H * HD], BF16)
    wqs_d = din("wqs", [P, DC, H * HD], BF16)
    wk_d = din("wk", [P, DC, KV * HD], BF16)
    wks_d = din("wks", [P, DC, KV * HD], BF16)
    wv_d = din("wv", [P, DC, KV * HD], BF16)
    piw_d = din("piw", [P, DC, D], BF16)
    gw_d = din("gw", [P, DC, D], BF16)
    pow_d = din("pow", [P, DC, D], BF16)
    wo_d = din("wo", [P, DC, D], BF16)
    lgr_d = din("lgr", [P, DC, D], BF16)
    lga_d = din("lga", [P, DC, D], BF16)
    lgw1_d = din("lgw1", [P, 2 * DC, D], BF16)
    lgw2_d = din("lgw2", [P, DC, D], BF16)
    rw1_d = din("rw1", [P, DC, HID], BF16)
    rw3_d = din("rw3", [P, DC, HID], BF16)
    rw2_d = din("rw2", [P, HC, D], BF16)
    cosq_d = din("cosq", [P, TO])
    sinq_d = din("sinq", [P, TO])
    cosk_d = din("cosk", [P, T])
    sink_d = din("sink", [P, T])
    pen_d = din("pen", [P, NCH])                  # exp bias per k-token
    tri_d = din("tri", [P, OC, TO], BF16)         # causal tri mask, own block
    umask_d = din("umask", [P, T], BF16)          # -30000 on future (replic.)
    gbc_d = din("gbc", [P, DC])                   # gate_b feature-major
    lgb1_d = din("lgb1", [P, DC])
    lng_d = din("lng", [P, DC])
    lnb_d = din("lnb", [P, DC])
    lgb2_d = din("lgb2", [P, DC])
    ffnw_d = din("ffnw", [P, DC])
    out_d = nc.dram_tensor("out", [P, DC, TO], F32, kind="ExternalOutput")

    def loadc(pool, name, dram, shape, dt=F32):
        t = pool.tile(shape, dt, name=name, bufs=1)
        nc.sync.dma_start(t[:], dram[:])
        return t

    def act_silu(out_ap, in_ap, tmp_pool):
        # CoreSim has no Silu; compose it there. HW uses the native LUT.
        if debug:
            t = tmp_pool.tile([in_ap.shape[0], in_ap.shape[-1]], F32,
                              tag="silu_t")
            nc.scalar.activation(t[:], in_ap, AF.Sigmoid)
            nc.vector.tensor_tensor(out_ap, in_ap, t[:], AL.mult)
        else:
            nc.scalar.activation(out_ap, in_ap, AF.Silu)

    with tile.TileContext(nc) as tc:
        with tc.tile_pool(name="cp", bufs=1) as cp, \
             tc.tile_pool(name="scol", bufs=1) as scol:
            gbc = loadc(cp, "gbc", gbc_d, [P, DC])
            lgb1 = loadc(cp, "lgb1", lgb1_d, [P, DC])
            lng = loadc(cp, "lng", lng_d, [P, DC])
            lnb = loadc(cp, "lnb", lnb_d, [P, DC])
            lgb2 = loadc(cp, "lgb2", lgb2_d, [P, DC])
            ffnw = loadc(cp, "ffnw", ffnw_d, [P, DC])
            onesp = cp.tile([P, 1], F32, name="onesp")
            nc.vector.memset(onesp[:], 1.0)
            ones_row = cp.tile([1, P], F32, name="ones_row")
            nc.vector.memset(ones_row[:], 1.0)
            eps_r = cp.tile([P, 1], F32, name="eps_r")
            nc.vector.memset(eps_r[:], EPS_RMS)
            eps_l = cp.tile([P, 1], F32, name="eps_l")
            nc.vector.memset(eps_l[:], EPS_LN)
            rstdh = scol.tile([1, TO], F32, name="rstdh")

            # right-side stacks: x2 (E..F) outer, attention operands inner
            with tc.tile_pool(name="px2", bufs=1, side="right") as px2, \
                 tc.tile_pool(name="patt", bufs=1, side="right") as patt:
                with tc.tile_pool(name="ph", bufs=1) as ph:
                    h_bf = ph.tile([P, DC, T], BF16, name="h_bf")

                    # ===== Phase A: h = rms_norm(x), feature-major ========
                    with tc.tile_pool(name="pa", bufs=2) as pa, \
                         tc.tile_pool(name="paps", bufs=1, space="PSUM") as paps:
                        ssq = [paps.tile([1, TO], F32, name=f"ssq{tq}")
                               for tq in range(4)]
                        for dc in range(DC):
                            xt = pa.tile([P, T], F32, tag="xt")
                            nc.sync.dma_start(xt[:], x_d[:, dc, :])
                            sq = pa.tile([P, T], F32, tag="sq")
                            nc.scalar.activation(sq[:], xt[:], AF.Square)
                            for tq in range(4):
                                nc.tensor.matmul(
                                    ssq[tq][:], onesp[:],
                                    sq[:, tq * TO:(tq + 1) * TO],
                                    start=(dc == 0), stop=(dc == DC - 1))
                        rstd = pa.tile([1, T], F32, name="rstd_x", bufs=1)
                        for tq in range(4):
                            sd = pa.tile([1, TO], F32, tag="sd")
                            nc.scalar.activation(sd[:], ssq[tq][:], AF.Sqrt,
                                                 bias=eps_r[0:1, :],
                                                 scale=1.0 / D)
                            nc.vector.reciprocal(
                                rstd[:, tq * TO:(tq + 1) * TO], sd[:])
                        rrep = paps.tile([P, T], F32, name="rrep")
                        for tq in range(4):
                            sl = slice(tq * TO, (tq + 1) * TO)
                            nc.tensor.matmul(rrep[:, sl], ones_row[:],
                                             rstd[:, sl], start=True, stop=True)
                        for dc in range(DC):
                            xt = pa.tile([P, T], F32, tag="xt")
                            nc.sync.dma_start(xt[:], x_d[:, dc, :])
                            nc.vector.tensor_tensor(h_bf[:, dc, :], xt[:],
                                                    rrep[:], AL.mult)

                    # ===== Phase B: q,k,v,u,pre + scan ====================
                    with tc.tile_pool(name="pbw", bufs=2) as pbw, \
                         tc.tile_pool(name="pbc", bufs=1) as pbc, \
                         tc.tile_pool(name="pbt", bufs=2) as pbt, \
                         tc.tile_pool(name="pbs", bufs=1) as pbs, \
                         tc.tile_pool(name="pbcs", bufs=2) as pbcs, \
                         tc.tile_pool(name="pbps", bufs=2, space="PSUM") as pbps, \
                         tc.tile_pool(name="pbps2", bufs=2, space="PSUM") as pbps2:
                        cosq = loadc(pbc, "cosq", cosq_d, [P, TO])
                        sinq = loadc(pbc, "sinq", sinq_d, [P, TO])
                        umask = loadc(pbc, "umask", umask_d, [P, T], BF16)
                        q_bf = patt.tile([P, DC, TO], BF16, name="q_bf")
                        k_bf = patt.tile([P, 2, T], BF16, name="k_bf")
                        v_aug = patt.tile([P, NCH, KV, HD + 1], BF16,
                                          name="v_aug")
                        hs_own = patt.tile([P, DC, TO], BF16, name="hs_own")
                        nc.vector.memset(v_aug[:, :, :, HD:HD + 1], 1.0)

                        # q + rope (own tokens = last TO of rotated h)
                        wq = pbw.tile([P, DC, H * HD], BF16, tag="w1k")
                        nc.sync.dma_start(wq[:], wq_d[:])
                        wqs = pbw.tile([P, DC, H * HD], BF16, tag="w1k")
                        nc.sync.dma_start(wqs[:], wqs_d[:])
                        for n in range(DC):
                            psq = pbps.tile([P, TO], F32, tag="psq")
                            psqs = pbps.tile([P, TO], F32, tag="psqs")
                            for dc in range(DC):
                                nc.tensor.matmul(
                                    psq[:], wq[:, dc, n * P:(n + 1) * P],
                                    h_bf[:, dc, T - TO:],
                                    start=(dc == 0), stop=(dc == DC - 1))
                            for dc in range(DC):
                                nc.tensor.matmul(
                                    psqs[:], wqs[:, dc, n * P:(n + 1) * P],
                                    h_bf[:, dc, T - TO:],
                                    start=(dc == 0), stop=(dc == DC - 1))
                            t1 = pbt.tile([P, TO], F32, tag="t1")
                            t2 = pbt.tile([P, TO], F32, tag="t2")
                            nc.vector.tensor_tensor(t1[:], psq[:], cosq[:],
                                                    AL.mult)
                            nc.vector.tensor_tensor(t2[:], psqs[:], sinq[:],
                                                    AL.mult)
                            nc.vector.tensor_tensor(q_bf[:, n, :], t1[:], t2[:],
                                                    AL.add)

                        # k + rope (full rotated seq; 1/sqrt(HD) pre-folded)
                        wk = pbw.tile([P, DC, KV * HD], BF16, tag="w256")
                        nc.sync.dma_start(wk[:], wk_d[:])
                        wks = pbw.tile([P, DC, KV * HD], BF16, tag="w256")
                        nc.sync.dma_start(wks[:], wks_d[:])
                        for n in range(2):
                            for ts in range(4):
                                sl = slice(ts * TO, (ts + 1) * TO)
                                ck = pbcs.tile([P, TO], F32, tag="ck")
                                nc.sync.dma_start(ck[:], cosk_d[:, sl])
                                sk = pbcs.tile([P, TO], F32, tag="sk")
                                nc.sync.dma_start(sk[:], sink_d[:, sl])
                                psk = pbps.tile([P, TO], F32, tag="psq")
                                psks = pbps.tile([P, TO], F32, tag="psqs")
                                for dc in range(DC):
                                    nc.tensor.matmul(
                                        psk[:], wk[:, dc, n * P:(n + 1) * P],
                                        h_bf[:, dc, sl],
                                        start=(dc == 0), stop=(dc == DC - 1))
                                for dc in range(DC):
                                    nc.tensor.matmul(
                                        psks[:], wks[:, dc, n * P:(n + 1) * P],
                                        h_bf[:, dc, sl],
                                        start=(dc == 0), stop=(dc == DC - 1))
                                t1 = pbt.tile([P, TO], F32, tag="t1")
                                t2 = pbt.tile([P, TO], F32, tag="t2")
                                nc.vector.tensor_tensor(t1[:], psk[:], ck[:],
                                                        AL.mult)
                                nc.vector.tensor_tensor(t2[:], psks[:], sk[:],
                                                        AL.mult)
                                nc.vector.tensor_tensor(k_bf[:, n, sl],
                                                        t1[:], t2[:], AL.add)

                        # v (token-major, stationary h chunks)
                        wv = pbw.tile([P, DC, KV * HD], BF16, tag="w256")
                        nc.sync.dma_start(wv[:], wv_d[:])
                        for c in range(NCH):
                            psv = pbps2.tile([P, KV * HD], F32, tag="psv")
                            for dc in range(DC):
                                nc.tensor.matmul(
                                    psv[:], h_bf[:, dc, c * P:(c + 1) * P],
                                    wv[:, dc, :],
                                    start=(dc == 0), stop=(dc == DC - 1))
                            nc.vector.tensor_copy(
                                v_aug[:, c, :, 0:HD],
                                psv.rearrange("p (k h) -> p k h", k=KV))

                        # u, pre -> f = sigmoid(pre), e = exp(u+umask), scan
                        piw = pbw.tile([P, DC, D], BF16, tag="w1k")
                        nc.sync.dma_start(piw[:], piw_d[:])
                        gw = pbw.tile([P, DC, D], BF16, tag="w1k")
                        nc.sync.dma_start(gw[:], gw_d[:])
                        ssqh = pbps2.tile([1, TO], F32, name="ssqh", bufs=1)
                        for n in range(DC):
                            f_sb = pbs.tile([P, T], F32, tag="f_sb")
                            e_sb = pbs.tile([P, T], F32, tag="e_sb")
                            scr = pbs.tile([P, T - TO], F32, tag="scr")
                            for ts in range(4):
                                sl = slice(ts * TO, (ts + 1) * TO)
                                psu = pbps.tile([P, TO], F32, tag="psq")
                                psp = pbps.tile([P, TO], F32, tag="psqs")
                                for dc in range(DC):
                                    nc.tensor.matmul(
                                        psu[:], piw[:, dc, n * P:(n + 1) * P],
                                        h_bf[:, dc, sl],
                                        start=(dc == 0), stop=(dc == DC - 1))
                                for dc in range(DC):
                                    nc.tensor.matmul(
                                        psp[:], gw[:, dc, n * P:(n + 1) * P],
                                        h_bf[:, dc, sl],
                                        start=(dc == 0), stop=(dc == DC - 1))
                                nc.scalar.activation(f_sb[:, sl], psp[:],
                                                     AF.Sigmoid,
                                                     bias=gbc[:, n:n + 1])
                                tu = pbt.tile([P, TO], F32, tag="t1")
                                nc.vector.tensor_tensor(tu[:], psu[:],
                                                        umask[:, sl], AL.add)
                                nc.scalar.activation(e_sb[:, sl], tu[:], AF.Exp)
                            nc.vector.tensor_tensor_scan(
                                scr[:], f_sb[:, :T - TO], e_sb[:, :T - TO],
                                0.0, AL.mult, AL.add)
                            nc.vector.tensor_tensor_scan(
                                hs_own[:, n, :], f_sb[:, T - TO:],
                                e_sb[:, T - TO:],
                                scr[:, T - TO - 1:T - TO], AL.mult, AL.add)
                            sqh = pbt.tile([P, TO], F32, tag="sqh", bufs=1)
                            nc.scalar.activation(sqh[:], hs_own[:, n, :],
                                                 AF.Square)
                            nc.tensor.matmul(ssqh[:], onesp[:], sqh[:],
                                             start=(n == 0), stop=(n == DC - 1))
                        sdh = pbt.tile([1, TO], F32, tag="sdh", bufs=1)
                        nc.scalar.activation(sdh[:], ssqh[:], AF.Sqrt,
                                             bias=eps_r[0:1, :], scale=1.0 / D)
                        nc.vector.reciprocal(rstdh[:], sdh[:])
                # h_bf freed here

                # ===== Phase C: attention + the four 1k projections =======
                with tc.tile_pool(name="pfg", bufs=1) as pfg:
                  with tc.tile_pool(name="pcb", bufs=1) as pcb, \
                     tc.tile_pool(name="pcw", bufs=2) as pcw, \
                     tc.tile_pool(name="pct", bufs=2) as pct, \
                     tc.tile_pool(name="pce", bufs=4) as pce, \
                     tc.tile_pool(name="pcps", bufs=2, space="PSUM") as pcps, \
                     tc.tile_pool(name="pcpo", bufs=2, space="PSUM") as pcpo, \
                     tc.tile_pool(name="pcps2", bufs=1, space="PSUM") as pcps2:
                    pen = loadc(pcb, "pen", pen_d, [P, NCH])
                    tri = loadc(pcb, "tri", tri_d, [P, OC, TO], BF16)
                    attn = pcb.tile([P, DC, TO], BF16, name="attn")
                    for hi in range(H):
                        kvh = hi // (H // KV)
                        kchunk, koff = kvh // 2, (kvh % 2) * HD
                        qchunk, qoff = (hi % 4) + 4 * (kvh // 2), koff
                        pout = pcpo.tile([HD + 1, TO], F32, tag="pout")
                        for kc in range(NCH):
                            ps = pcps.tile([P, TO], F32, tag="ps")
                            nc.tensor.matmul(
                                ps[:],
                                k_bf[koff:koff + HD, kchunk,
                                     kc * P:(kc + 1) * P],
                                q_bf[qoff:qoff + HD, qchunk, :],
                                start=True, stop=True)
                            ea = pce.tile([P, TO], BF16, tag="ea")
                            nc.scalar.activation(ea[:], ps[:], AF.Exp,
                                                 bias=pen[:, kc:kc + 1])
                            if kc >= NCH - OC:
                                nc.vector.tensor_tensor(
                                    ea[:], ea[:], tri[:, kc - (NCH - OC), :],
                                    AL.mult)
                            nc.tensor.matmul(
                                pout[:], v_aug[:, kc, kvh, :], ea[:],
                                start=(kc == 0), stop=(kc == NCH - 1))
                        zrow = pct.tile([1, TO], F32, tag="zrow")
                        nc.vector.tensor_copy(zrow[:], pout[HD:HD + 1, :])
                        zrep = pcps2.tile([HD, TO], F32, tag="zrep")
                        nc.tensor.matmul(zrep[:], ones_row[:, :HD], zrow[:],
                                         start=True, stop=True)
                        zinv = pce.tile([HD, TO], F32, tag="zinv")
                        nc.vector.reciprocal(zinv[:], zrep[:])
                        nc.vector.tensor_tensor(
                            attn[qoff:qoff + HD, qchunk, :],
                            pout[0:HD, :], zinv[:], AL.mult)

                    # out_local / out_global (bf16) ; lg projections
                    hsn = pcb.tile([P, DC, TO], BF16, name="hsn")
                    ol_b = pfg.tile([P, DC, TO], BF16, name="ol_b")
                    og_b = pfg.tile([P, DC, TO], BF16, name="og_b")
                    hr = pfg.tile([P, DC, TO], BF16, name="hr")
                    ha = pfg.tile([P, DC, TO], BF16, name="ha")
                    hrep = pcps2.tile([P, TO], F32, name="hrep", bufs=1)
                    nc.tensor.matmul(hrep[:], ones_row[:], rstdh[:],
                                     start=True, stop=True)
                    for n in range(DC):
                        nc.vector.tensor_tensor(hsn[:, n, :], hs_own[:, n, :],
                                                hrep[:], AL.mult)

                    def proj(w_dram, rhs, ot):
                        w = pcw.tile([P, DC, D], BF16, tag="w1k")
                        nc.sync.dma_start(w[:], w_dram[:])
                        for n in range(DC):
                            pso = pcps.tile([P, TO], F32, tag="ps")
                            for dc in range(DC):
                                nc.tensor.matmul(
                                    pso[:], w[:, dc, n * P:(n + 1) * P],
                                    rhs[:, dc, :],
                                    start=(dc == 0), stop=(dc == DC - 1))
                            nc.vector.tensor_copy(ot[:, n, :], pso[:])

                    proj(wo_d, attn, ol_b)
                    proj(pow_d, hsn, og_b)
                    proj(lgr_d, og_b, hr)
                    proj(lga_d, ol_b, ha)

                  # ===== Phase E: logic gate + mixing =====================
                  with tc.tile_pool(name="peb", bufs=1) as peb, \
                       tc.tile_pool(name="pet", bufs=2) as pet, \
                       tc.tile_pool(name="peps", bufs=2, space="PSUM") as peps, \
                       tc.tile_pool(name="peps2", bufs=1,
                                    space="PSUM") as peps2:
                        g_sb = peb.tile([P, DC, TO], F32, name="g_sb")
                        lgw1 = peb.tile([P, 2 * DC, D], BF16, tag="wlg")
                        nc.sync.dma_start(lgw1[:], lgw1_d[:])
                        mu_ps = peps2.tile([1, TO], F32, name="mu_ps")
                        ss_ps = peps2.tile([1, TO], F32, name="ss_ps")
                        for n in range(DC):
                            psg = peps.tile([P, TO], F32, tag="ps")
                            for dc in range(DC):
                                nc.tensor.matmul(
                                    psg[:], lgw1[:, dc, n * P:(n + 1) * P],
                                    hr[:, dc, :], start=(dc == 0), stop=False)
                            for dc in range(DC):
                                nc.tensor.matmul(
                                    psg[:], lgw1[:, DC + dc, n * P:(n + 1) * P],
                                    ha[:, dc, :], start=False,
                                    stop=(dc == DC - 1))
                            nc.vector.tensor_scalar(g_sb[:, n, :], psg[:],
                                                    lgb1[:, n:n + 1], None,
                                                    AL.add)
                            gsq = pet.tile([P, TO], F32, tag="gsq")
                            nc.scalar.activation(gsq[:], g_sb[:, n, :],
                                                 AF.Square)
                            nc.tensor.matmul(mu_ps[:], onesp[:], g_sb[:, n, :],
                                             start=(n == 0), stop=(n == DC - 1))
                            nc.tensor.matmul(ss_ps[:], onesp[:], gsq[:],
                                             start=(n == 0), stop=(n == DC - 1))
                        mu = scol.tile([1, TO], F32, name="mu")
                        nc.vector.tensor_scalar(mu[:], mu_ps[:], 1.0 / D, None,
                                                AL.mult)
                        musq = scol.tile([1, TO], F32, name="musq")
                        nc.vector.tensor_tensor(musq[:], mu[:], mu[:], AL.mult)
                        var = scol.tile([1, TO], F32, name="var")
                        nc.vector.scalar_tensor_tensor(
                            var[:], ss_ps[:], 1.0 / D, musq[:],
                            AL.mult, AL.subtract)
                        sdg = scol.tile([1, TO], F32, name="sdg")
                        nc.scalar.activation(sdg[:], var[:], AF.Sqrt,
                                             bias=eps_l[0:1, :])
                        rstdg = scol.tile([1, TO], F32, name="rstdg")
                        nc.vector.reciprocal(rstdg[:], sdg[:])
                        murep = peps2.tile([P, TO], F32, name="murep")
                        nc.tensor.matmul(murep[:], ones_row[:], mu[:],
                                         start=True, stop=True)
                        grep = peps2.tile([P, TO], F32, name="grep")
                        nc.tensor.matmul(grep[:], ones_row[:], rstdg[:],
                                         start=True, stop=True)
                        sg = peb.tile([P, DC, TO], BF16, name="sg")
                        for n in range(DC):
                            tg = pet.tile([P, TO], F32, tag="tg")
                            nc.vector.tensor_tensor(tg[:], g_sb[:, n, :],
                                                    murep[:], AL.subtract)
                            nc.vector.tensor_tensor(tg[:], tg[:], grep[:],
                                                    AL.mult)
                            nc.vector.tensor_scalar(tg[:], tg[:],
                                                    lng[:, n:n + 1],
                                                    lnb[:, n:n + 1],
                                                    AL.mult, AL.add)
                            act_silu(sg[:, n, :], tg[:], pet)

                        x2 = px2.tile([P, DC, TO], F32, name="x2")
                        xqt = peb.tile([P, DC, TO], F32, name="xqt")
                        nc.sync.dma_start(xqt[:], xq_d[:])
                        lgw2 = peb.tile([P, DC, D], BF16, tag="wlg")
                        nc.sync.dma_start(lgw2[:], lgw2_d[:])
                        for n in range(DC):
                            psg = peps.tile([P, TO], F32, tag="ps")
                            for dc in range(DC):
                                nc.tensor.matmul(
                                    psg[:], lgw2[:, dc, n * P:(n + 1) * P],
                                    sg[:, dc, :],
                                    start=(dc == 0), stop=(dc == DC - 1))
                            gate = pet.tile([P, TO], F32, tag="gate")
                            nc.scalar.activation(gate[:], psg[:], AF.Sigmoid,
                                                 bias=lgb2[:, n:n + 1])
                            dm = pet.tile([P, TO], F32, tag="tg")
                            nc.vector.tensor_tensor(dm[:], ol_b[:, n, :],
                                                    og_b[:, n, :], AL.subtract)
                            nc.vector.tensor_tensor(dm[:], dm[:], gate[:],
                                                    AL.mult)
                            nc.vector.tensor_tensor(dm[:], dm[:], og_b[:, n, :],
                                                    AL.add)
                            nc.vector.tensor_tensor(x2[:, n, :], dm[:],
                                                    xqt[:, n, :], AL.add)

            # ================= Phase F: recursive gated FFN ===============
            with tc.tile_pool(name="pf", bufs=2) as pf, \
                 tc.tile_pool(name="pfw", bufs=2) as pfw, \
                 tc.tile_pool(name="pfc", bufs=1) as pfc, \
                 tc.tile_pool(name="pfps", bufs=2, space="PSUM") as pfps, \
                 tc.tile_pool(name="pfps2", bufs=1, space="PSUM") as pfps2:
                state = pfc.tile([P, DC, TO], F32, name="state")
                s_bf = pfc.tile([P, DC, TO], BF16, name="s_bf")
                c_bf = pfc.tile([P, HC, TO], BF16, name="c_bf")

                def rms_rep(src):
                    ssp = pfps2.tile([1, TO], F32, tag="ssp")
                    for n in range(DC):
                        sq = pf.tile([P, TO], F32, tag="sq")
                        nc.scalar.activation(sq[:], src[:, n, :], AF.Square)
                        nc.tensor.matmul(ssp[:], onesp[:], sq[:],
                                         start=(n == 0), stop=(n == DC - 1))
                    sd = pf.tile([1, TO], F32, tag="sd1")
                    nc.scalar.activation(sd[:], ssp[:], AF.Sqrt,
                                         bias=eps_r[0:1, :], scale=1.0 / D)
                    rs = pf.tile([1, TO], F32, tag="rs1")
                    nc.vector.reciprocal(rs[:], sd[:])
                    rrep = pfps2.tile([P, TO], F32, tag="rrepf")
                    nc.tensor.matmul(rrep[:], ones_row[:], rs[:],
                                     start=True, stop=True)
                    return rrep

                rrep = rms_rep(x2)
                for n in range(DC):
                    tg = pf.tile([P, TO], F32, tag="tg")
                    nc.vector.tensor_tensor(tg[:], x2[:, n, :], rrep[:],
                                            AL.mult)
                    nc.vector.tensor_scalar(state[:, n, :], tg[:],
                                            ffnw[:, n:n + 1], None, AL.mult)

                for it in range(DEPTH):
                    rrep = rms_rep(state)
                    for n in range(DC):
                        nc.vector.tensor_tensor(s_bf[:, n, :], state[:, n, :],
                                                rrep[:], AL.mult)
                    rw1 = pfw.tile([P, DC, HID], BF16, tag="w3k")
                    nc.sync.dma_start(rw1[:], rw1_d[:])
                    rw3 = pfw.tile([P, DC, HID], BF16, tag="w3k")
                    nc.sync.dma_start(rw3[:], rw3_d[:])
                    for hc in range(HC):
                        psa = pfps.tile([P, TO], F32, tag="psa")
                        psb = pfps.tile([P, TO], F32, tag="psb")
                        for dc in range(DC):
                            nc.tensor.matmul(
                                psa[:], rw1[:, dc, hc * P:(hc + 1) * P],
                                s_bf[:, dc, :],
                                start=(dc == 0), stop=(dc == DC - 1))
                        for dc in range(DC):
                            nc.tensor.matmul(
                                psb[:], rw3[:, dc, hc * P:(hc + 1) * P],
                                s_bf[:, dc, :],
                                start=(dc == 0), stop=(dc == DC - 1))
                        asl = pf.tile([P, TO], BF16, tag="asl")
                        act_silu(asl[:], psa[:], pf)
                        nc.vector.tensor_tensor(c_bf[:, hc, :], asl[:], psb[:],
                                                AL.mult)
                    rw2 = pfw.tile([P, HC, D], BF16, tag="w3k")
                    nc.sync.dma_start(rw2[:], rw2_d[:])
                    for n in range(DC):
                        psd = pfps.tile([P, TO], F32, tag="psa")
                        for hc in range(HC):
                            nc.tensor.matmul(
                                psd[:], rw2[:, hc, n * P:(n + 1) * P],
                                c_bf[:, hc, :],
                                start=(hc == 0), stop=(hc == HC - 1))
                        nc.vector.tensor_tensor(state[:, n, :], state[:, n, :],
                                                psd[:], AL.add)
                for n in range(DC):
                    ot = pf.tile([P, TO], F32, tag="tg")
                    nc.vector.tensor_tensor(ot[:], x2[:, n, :], state[:, n, :],
                                            AL.add)
                    nc.sync.dma_start(out_d[:, n, :], ot[:])

    nc.compile()
    return nc


# ==================== host-side input preparation ====================

def _bf(a):
    return np.ascontiguousarray(np.asarray(a, np.float32)).astype(
        ml_dtypes.bfloat16)


def _fm(w):
    """[D_in, N] weight -> feature-major chunks [P, DC_in, N]."""
    din = w.shape[0]
    return np.ascontiguousarray(
        w.reshape(din // P, P, -1).transpose(1, 0, 2))


def _rope_perm(nheads, swapped):
    idx = []
    for h in range(nheads):
        base = h * HD
        a = base + np.arange(0, HD, 2)
        b = base + np.arange(1, HD, 2)
        idx.extend((b if swapped else a))
        idx.extend((a if swapped else b))
    return np.array(idx)


def _q_row_of_head(hi):
    kvh = hi // 4
    qchunk = (hi % 4) + 4 * (kvh // 2)
    qoff = (kvh % 2) * HD
    return qchunk * P + qoff


def _q_layout_perm(swapped):
    idx = np.zeros(H * HD, np.int64)
    for hi in range(H):
        r = _q_row_of_head(hi)
        a = hi * HD + np.arange(0, HD, 2)
        b = hi * HD + np.arange(1, HD, 2)
        first, second = (b, a) if swapped else (a, b)
        idx[r:r + 32] = first
        idx[r + 32:r + 64] = second
    return idx


def _wo_row_perm():
    idx = np.zeros(H * HD, np.int64)
    for hi in range(H):
        r = _q_row_of_head(hi)
        idx[r:r + HD] = hi * HD + np.arange(HD)
    return idx


def _rep_cos(c):
    ct = np.ascontiguousarray(np.asarray(c, np.float32).T)  # [32, Tn]
    return np.ascontiguousarray(np.tile(ct, (4, 1)))


def _rep_sin_pm(s):
    st = np.ascontiguousarray(np.asarray(s, np.float32).T)  # [32, Tn]
    return np.ascontiguousarray(np.concatenate([-st, st, -st, st], axis=0))


def _prep_core(inputs, b, j, shared):
    c0, c1 = j * TO, (j + 1) * TO
    x = np.asarray(inputs["x"][b], np.float32)
    cos = np.asarray(inputs["freqs_cos"], np.float32)
    sin = np.asarray(inputs["freqs_sin"], np.float32)
    rot = np.concatenate([np.arange(c1, T), np.arange(0, c1)])
    n_fut = T - c1

    m = dict(shared)
    m["x"] = _fm(np.ascontiguousarray(x[rot].T).reshape(D, T))
    m["xq"] = _fm(np.ascontiguousarray(x[c0:c1].T).reshape(D, TO))
    m["cosq"] = _rep_cos(cos[c0:c1])
    m["sinq"] = _rep_sin_pm(sin[c0:c1])
    m["cosk"] = _rep_cos(cos[rot])
    m["sink"] = _rep_sin_pm(sin[rot])
    pen = np.zeros(T, np.float32)
    pen[:n_fut] = PEN
    m["pen"] = np.ascontiguousarray(pen.reshape(NCH, P).T)
    um = np.zeros((P, T), np.float32)
    um[:, :n_fut] = PEN
    m["umask"] = um.astype(ml_dtypes.bfloat16)
    return m


def _prep_shared(inputs):
    f32 = lambda k: np.asarray(inputs[k], np.float32)
    anw = f32("attn_norm_w")
    ksc = 1.0 / np.sqrt(HD)

    wq = f32("wq") * anw[:, None]
    wk = f32("wk") * anw[:, None] * ksc
    s = {}
    s["wq"] = _bf(_fm(wq[:, _q_layout_perm(False)]))
    s["wqs"] = _bf(_fm(wq[:, _q_layout_perm(True)]))
    s["wk"] = _bf(_fm(wk[:, _rope_perm(KV, False)]))
    s["wks"] = _bf(_fm(wk[:, _rope_perm(KV, True)]))
    s["wv"] = _bf(_fm(f32("wv") * anw[:, None]))
    s["piw"] = _bf(_fm(f32("proj_in_w")[:, :D] * anw[:, None]))
    s["gw"] = _bf(_fm(f32("gate_w") * anw[:, None]))
    s["pow"] = _bf(_fm(f32("proj_out_w") * f32("c_norm_w")[:, None]))
    s["wo"] = _bf(_fm(f32("wo")[_wo_row_perm(), :]))
    s["lgr"] = _bf(_fm(f32("lg_proj_rnn")))
    s["lga"] = _bf(_fm(f32("lg_proj_attn")))
    s["lgw1"] = _bf(_fm(f32("lg_w1")))
    s["lgw2"] = _bf(_fm(f32("lg_w2")))
    rnw = f32("rs_norm_w")
    s["rw1"] = _bf(_fm(f32("rs_w1") * rnw[:, None]))
    s["rw3"] = _bf(_fm(f32("rs_w3") * rnw[:, None]))
    s["rw2"] = _bf(_fm(f32("rs_w2")))

    tri = np.zeros((P, OC, TO), np.float32)
    for kc in range(OC):
        kpos = kc * P + np.arange(P)
        tri[:, kc, :] = (kpos[:, None] <= np.arange(TO)[None, :])
    s["tri"] = tri.astype(ml_dtypes.bfloat16)

    def col(k):
        return np.ascontiguousarray(f32(k).reshape(DC, P).T)

    s["gbc"] = col("gate_b")
    s["lgb1"] = col("lg_b1")
    s["lng"] = col("lg_ln_g")
    s["lnb"] = col("lg_ln_b")
    s["lgb2"] = col("lg_b2")
    s["ffnw"] = col("ffn_norm_w")
    return s


def _get_nc():
    if "nc" not in _CACHE:
        _CACHE["nc"] = _build(debug=False)
    return _CACHE["nc"]


def make_in_maps(inputs):
    shared = _prep_shared(inputs)
    return [_prep_core(inputs, core // 4, core % 4, shared)
            for core in range(8)]


def assemble(results):
    out = np.zeros((B, T, D), np.float32)
    for core in range(8):
        b, j = core // 4, core % 4
        o = np.asarray(results[core]["out"], np.float32)  # [P, DC, TO]
        out[b, j * TO:(j + 1) * TO] = o.transpose(1, 0, 2).reshape(D, TO).T
    return out


def run(inputs, trace=False, tmpdir=None):
    nc = _get_nc()
    in_maps = make_in_maps(inputs)
    res = run_bass_kernel_spmd(nc, in_maps, list(range(8)), trace=trace,
                               tmpdir=tmpdir)
    return assemble(res.results), res


def kernel(**inputs):
    out, _ = run(inputs, trace=False)
    return out.astype(np.asarray(inputs["x"]).dtype)
